# revision 20
# baseline (speedup 1.0000x reference)
"""Multi-head self-attention with SDPA softcap, sharded over 8 NeuronCores.

Sharding: tensor-parallel over heads. Each core owns 2 of the 16 heads.
Single fused pass: the q/k/v projections for batch 1 are interleaved with
attention units of batch 0, so the activation/vector engines (softmax
tanh+exp, bias adds) stay busy under the projection matmuls.

Dtypes: fp16 for x/weights/q/k/bias/s (11-bit mantissa keeps softmax
logits accurate), bf16 for v and exp(scores) (range), fp32 PSUM/output.
"""

import sys

if "/opt/trn_rl_repo" not in sys.path:
    sys.path.insert(0, "/opt/trn_rl_repo")

import numpy as np

import concourse.bass as bass
import concourse.bacc as bacc
import concourse.tile as tile
from concourse import bass_isa, mybir
from concourse.bass_utils import run_bass_kernel_spmd

F32 = mybir.dt.float32
F32R = mybir.dt.float32r
BF16 = mybir.dt.bfloat16
F16 = mybir.dt.float16

D = 2048          # model dim
H = 16            # total heads
DK = 128          # head dim
B = 2
S = 2048
T = B * S         # 4096 total tokens
NCORES = 8
HC = 2            # heads per core
DPC = HC * DK     # 256: d' slice per core

KC = D // 128     # 16 contraction chunks over model dim
KH = KC // 2      # 8: half of the contraction chunks
TCOL = 512        # phase-1 token-column width
NTCOL = T // TCOL             # 8
TQ = 256          # query-column width per attention unit
NTQ = S // TQ                 # 8 per batch
NTK = S // 128    # 16 key blocks per batch
NBT = T // 128    # 32 token blocks total
NHF = NTK // 2    # 8: half of the key blocks


def _build_program(cap: float):
    nc = bacc.Bacc("TRN2", target_bir_lowering=False, debug=False,
                   num_devices=NCORES)

    xT = nc.dram_tensor("xT", [D, T], F16, kind="ExternalInput").ap()
    ones_d = nc.dram_tensor("ones", [128, 128], BF16, kind="ExternalInput").ap()
    wqT = nc.dram_tensor("wqT", [D, DPC], F16, kind="ExternalInput").ap()
    wkT = nc.dram_tensor("wkT", [D, DPC], F16, kind="ExternalInput").ap()
    wvT = nc.dram_tensor("wvT", [D, DPC], F16, kind="ExternalInput").ap()
    woT = nc.dram_tensor("woT", [DPC, D], F16, kind="ExternalInput").ap()
    biasT = nc.dram_tensor("biasT", [S, S], F16, kind="ExternalInput").ap()
    out_d = nc.dram_tensor("out_partial", [T, D], F32, kind="ExternalOutput").ap()

    xT_v = xT.rearrange("(kc p) t -> p kc t", p=128)
    biasT_v = biasT.rearrange("(kc p) t -> p kc t", p=128)

    with tile.TileContext(nc) as tc:
        with (
            tc.tile_pool(name="const", bufs=1) as cpool,
            tc.tile_pool(name="pqkv", bufs=1) as pqkv,
            tc.tile_pool(name="pwo", bufs=1) as pwo,
            tc.tile_pool(name="pbias", bufs=3) as pbias,
            tc.tile_pool(name="p1w", bufs=1) as p1w,
            tc.tile_pool(name="p1x", bufs=3) as p1x,
            tc.tile_pool(name="p2s", bufs=3) as p2s,
            tc.tile_pool(name="p2er", bufs=3) as p2er,
            tc.tile_pool(name="p2rec", bufs=2) as p2rec,
            tc.tile_pool(name="p2tr", bufs=2) as p2tr,
            tc.tile_pool(name="p2ot", bufs=6) as p2ot,
            tc.tile_pool(name="p2out", bufs=4) as p2out,
            tc.tile_pool(name="psps", bufs=2, space="PSUM") as psps,
            tc.tile_pool(name="pacc", bufs=2, space="PSUM") as pacc,
            tc.tile_pool(name="pio", bufs=2, space="PSUM") as pio,
        ):
            ones_sb = cpool.tile([128, 128], BF16)
            # q/k stored transposed per head: [dk, tokens]; v natural:
            # [token-block, token%128, (h dk)]
            q_sb = pqkv.tile([128, HC, T], F16)
            k_sb = pqkv.tile([128, HC, T], F16)
            v_sb = pqkv.tile([128, NBT, DPC], BF16)
            wo_sb = pwo.tile([128, HC, 4, 512], F16)

            wq_sb = p1w.tile([128, KC, DPC], F16)
            wk_sb = p1w.tile([128, KC, DPC], F16)
            wv_sb = p1w.tile([128, KC, DPC], F16)
            wqv = wqT.rearrange("(kc p) n -> p kc n", p=128)
            nc.sync.dma_start(out=wq_sb[:, 0:KH, :], in_=wqv[:, 0:KH, :])
            nc.scalar.dma_start(out=wq_sb[:, KH:KC, :], in_=wqv[:, KH:KC, :])
            first_x = []
            t0 = 0
            xa0 = p1x.tile([128, KH, TCOL], F16, tag="x")
            xb0 = p1x.tile([128, KH, TCOL], F16, tag="x")
            nc.sync.dma_start(out=xa0[:], in_=xT_v[:, 0:KH, 0:TCOL])
            nc.scalar.dma_start(out=xb0[:], in_=xT_v[:, KH:KC, 0:TCOL])
            first_x.append((xa0, xb0))
            nc.gpsimd.dma_start(
                out=wk_sb[:], in_=wkT.rearrange("(kc p) n -> p kc n", p=128))
            nc.gpsimd.dma_start(
                out=wv_sb[:], in_=wvT.rearrange("(kc p) n -> p kc n", p=128))
            nc.gpsimd.dma_start(out=ones_sb[:], in_=ones_d[:])

            bias_tiles = {}

            def load_bias(b, tqc):
                key = (b, tqc)
                if key in bias_tiles or tqc >= NTQ or b >= B:
                    return
                bt = pbias.tile([128, NTK, TQ], F16, tag="bias")
                nc.gpsimd.dma_start(
                    out=bt[:], in_=biasT_v[:, :, tqc * TQ:(tqc + 1) * TQ])
                bias_tiles[key] = bt

            ncopy = 0

            def proj_tcol(tcol):
                """Emit q/k/v projection work for one 512-token column."""
                nonlocal ncopy
                t0 = tcol * TCOL
                if first_x:
                    xa, xb = first_x.pop()
                else:
                    xa = p1x.tile([128, KH, TCOL], F16, tag="x")
                    xb = p1x.tile([128, KH, TCOL], F16, tag="x")
                    nc.sync.dma_start(
                        out=xa[:], in_=xT_v[:, 0:KH, t0:t0 + TCOL])
                    nc.scalar.dma_start(
                        out=xb[:], in_=xT_v[:, KH:KC, t0:t0 + TCOL])

                # q/k: stationary weights, transposed output [dk, tokens]
                for m in range(4):
                    wsb = wq_sb if m < 2 else wk_sb
                    msl = m % 2
                    dst = q_sb if m < 2 else k_sb
                    ps = pio.tile([128, TCOL], F32, tag="io", name="pqk")
                    for kc in range(KC):
                        xc = xa if kc < KH else xb
                        nc.tensor.matmul(
                            ps[:],
                            wsb[:, kc, msl * 128:(msl + 1) * 128],
                            xc[:, kc % KH, :],
                            start=(kc == 0),
                            stop=(kc == KC - 1),
                        )
                    if ncopy % 2 == 0:
                        nc.scalar.copy(dst[:, msl, t0:t0 + TCOL], ps[:])
                    else:
                        nc.vector.tensor_copy(dst[:, msl, t0:t0 + TCOL], ps[:])
                    ncopy += 1
                # v: stationary x chunks -> natural [t, (h d')] layout
                for tsub in range(TCOL // 128):
                    pv = pacc.tile([128, DPC], F32, tag="acc", name="pv")
                    for kc in range(KC):
                        xc = xa if kc < KH else xb
                        nc.tensor.matmul(
                            pv[:],
                            xc[:, kc % KH, tsub * 128:(tsub + 1) * 128],
                            wv_sb[:, kc, :],
                            start=(kc == 0),
                            stop=(kc == KC - 1),
                        )
                    if ncopy % 2 == 0:
                        nc.scalar.copy(
                            v_sb[:, tcol * (TCOL // 128) + tsub, :], pv[:])
                    else:
                        nc.vector.tensor_copy(
                            v_sb[:, tcol * (TCOL // 128) + tsub, :], pv[:])
                    ncopy += 1

            # units: batch-major so batch-0 units can interleave with the
            # batch-1 projection columns.
            units = [(b, tqc, h)
                     for b in range(B)
                     for tqc in range(NTQ)
                     for h in range(HC)]
            s_map = {}
            fin_state = {}
            ot_map = {}
            nout = 0

            def stage_a(i):
                b, tqc, h = units[i]
                if h == 0 and i + 4 < len(units):
                    load_bias(units[i + 4][0], units[i + 4][1])
                bt = bias_tiles[(b, tqc)]
                q0 = tqc * TQ
                qcol = q_sb[:, h, b * S + q0:b * S + q0 + TQ]
                s_buf = p2s.tile([128, NTK, TQ], F16, tag="s")
                for g in range(NTK // 4):
                    sps = psps.tile([128, 4, TQ], F32, tag="sps")
                    for j in range(4):
                        tkb = g * 4 + j
                        nc.tensor.matmul(
                            sps[:, j, :],
                            k_sb[:, h, b * S + tkb * 128:
                                 b * S + (tkb + 1) * 128],
                            qcol,
                            start=True,
                            stop=True,
                        )
                    nc.vector.tensor_add(
                        s_buf[:, g * 4:(g + 1) * 4, :],
                        sps[:],
                        bt[:, g * 4:(g + 1) * 4, :],
                    )
                s_map[i] = s_buf

            def stage_b(i):
                b, tqc, h = units[i]
                s_buf = s_map.pop(i)
                er = p2er.tile([128, NTK, TQ], BF16, tag="er")
                av = pacc.tile([128, TQ], F32, tag="acc", name="av")
                s_flat = s_buf[:].rearrange("p a b -> p (a b)")
                nc.scalar.activation(
                    s_flat, s_flat,
                    mybir.ActivationFunctionType.Tanh,
                    scale=1.0 / cap,
                )
                nc.scalar.activation(
                    er[:].rearrange("p a b -> p (a b)"),
                    s_flat,
                    mybir.ActivationFunctionType.Exp,
                    scale=cap,
                )
                for tkb in range(NTK):
                    nc.tensor.matmul(
                        av[:],
                        v_sb[:, b * NTK + tkb, h * DK:(h + 1) * DK],
                        er[:, tkb, :],
                        start=(tkb == 0),
                        stop=(tkb == NTK - 1),
                    )
                # softmax denominator: log-tree over key blocks (DVE),
                # then partition reduction on gpsimd
                r1 = p2tr.tile([128, NHF, TQ], BF16, tag="r1")
                nc.vector.tensor_add(r1[:], er[:, 0:NHF, :], er[:, NHF:NTK, :])
                r2 = p2tr.tile([128, 4, TQ], BF16, tag="r2")
                nc.vector.tensor_add(r2[:], r1[:, 0:4, :], r1[:, 4:8, :])
                r3 = p2tr.tile([128, 2, TQ], BF16, tag="r3")
                nc.vector.tensor_add(r3[:], r2[:, 0:2, :], r2[:, 2:4, :])
                zin = p2tr.tile([128, TQ], F32, tag="zin")
                nc.vector.tensor_add(zin[:], r3[:, 0, :], r3[:, 1, :])
                z = p2tr.tile([128, TQ], F32, tag="z")
                nc.gpsimd.partition_all_reduce(
                    z[:], zin[:], 128, bass_isa.ReduceOp.add)
                fin_state[i] = (av, z)

            def stage_b_fin(i):
                b, tqc, h = units[i]
                av, z = fin_state.pop(i)
                rec = p2rec.tile([128, TQ], F32, tag="rec")
                nc.vector.reciprocal_approx_fast(out=rec[:], in_=z[:])
                ot = p2ot.tile([128, TQ], F16, tag="ot")
                nc.vector.tensor_mul(ot[:], av[:], rec[:])
                ot_map[(b, tqc, h)] = ot

            def phase3(b, tqc):
                nonlocal nout
                o0 = ot_map.pop((b, tqc, 0))
                o1 = ot_map.pop((b, tqc, 1))
                for tb4 in range(TQ // 128):
                    trow = b * S + (tqc * (TQ // 128) + tb4) * 128
                    for ng in range(4):
                        po = pio.tile([128, 512], F32, tag="io", name="po")
                        for hc, o in ((0, o0), (1, o1)):
                            nc.tensor.matmul(
                                po[:],
                                o[:, tb4 * 128:(tb4 + 1) * 128],
                                wo_sb[:, hc, ng, :],
                                start=(hc == 0),
                                stop=(hc == HC - 1),
                            )
                        outt = p2out.tile([128, 512], F32, tag="outt")
                        if nout % 2 == 0:
                            nc.vector.tensor_copy(outt[:], po[:])
                        else:
                            nc.scalar.copy(outt[:], po[:])
                        nout += 1
                        nc.sync.dma_start(
                            out=out_d[trow:trow + 128,
                                      ng * 512:(ng + 1) * 512],
                            in_=outt[:],
                        )

            # ---------------- fused emission schedule --------------------
            steps_done = 0

            def unit_steps(n):
                """Advance the unit pipeline by n pipeline steps."""
                nonlocal steps_done
                for _ in range(n):
                    i = steps_done
                    if i >= len(units):
                        return
                    if i == 0:
                        stage_a(0)
                        stage_a(1)
                    stage_b(i)
                    if i + 2 < len(units):
                        stage_a(i + 2)
                    stage_b_fin(i)
                    b, tqc, h = units[i]
                    if h == 1:
                        phase3(b, tqc)
                    steps_done += 1

            proj_tcol(0)
            load_bias(0, 0)
            proj_tcol(1)
            load_bias(0, 1)
            proj_tcol(2)
            nc.gpsimd.dma_start(
                out=wo_sb[:],
                in_=woT.rearrange("(hc p) (ng n) -> p hc ng n", p=128, n=512),
            )
            proj_tcol(3)
            for tcol in range(4, NTCOL):
                proj_tcol(tcol)
                unit_steps(4)
            unit_steps(len(units) - steps_done)

    nc.compile()
    return nc


_PROGRAM_CACHE: dict = {}


def _get_program(cap: float):
    if cap not in _PROGRAM_CACHE:
        _PROGRAM_CACHE[cap] = _build_program(cap)
    return _PROGRAM_CACHE[cap]


def _prepare_in_maps(inp, wq, wk, wv, wo, attn_bias, softcap):
    x = np.ascontiguousarray(np.asarray(inp, dtype=np.float32)).reshape(T, D)
    xT = np.ascontiguousarray(x.T).astype(np.float16)
    biasT = np.ascontiguousarray(
        np.asarray(attn_bias, dtype=np.float32).reshape(S, S).T
    ).astype(np.float16)
    wq = np.asarray(wq, dtype=np.float32)
    wk = np.asarray(wk, dtype=np.float32)
    wv = np.asarray(wv, dtype=np.float32)
    wo = np.asarray(wo, dtype=np.float32)
    scale = 1.0 / np.sqrt(np.float32(DK))
    import ml_dtypes
    ones = np.ones((128, 128), dtype=np.float32).astype(ml_dtypes.bfloat16)

    in_maps = []
    for c in range(NCORES):
        rows = slice(c * DPC, (c + 1) * DPC)
        in_maps.append({
            "xT": xT,
            "ones": ones,
            "wqT": (wq[rows] * scale).T.astype(np.float16),
            "wkT": np.ascontiguousarray(wk[rows].T).astype(np.float16),
            "wvT": np.ascontiguousarray(wv[rows].T).astype(np.float16),
            "woT": np.ascontiguousarray(wo[:, rows].T).astype(np.float16),
            "biasT": biasT,
        })
    return in_maps


def run(inputs: dict, trace: bool = False):
    """Run the SPMD kernel. Returns (full_output, BassKernelResults)."""
    cap = float(inputs["softcap"])
    nc = _get_program(cap)
    in_maps = _prepare_in_maps(
        inputs["inp"], inputs["wq"], inputs["wk"], inputs["wv"],
        inputs["wo"], inputs["attn_bias"], inputs["softcap"],
    )
    res = run_bass_kernel_spmd(
        nc, in_maps, list(range(NCORES)), trace=trace,
    )
    acc = np.zeros((T, D), dtype=np.float32)
    for c in range(NCORES):
        acc += res.results[c]["out_partial"]
    out = acc.reshape(B, S, D)
    return out, res


def kernel(**inputs) -> np.ndarray:
    out, _ = run(inputs, trace=False)
    return out


if __name__ == "__main__":
    rng = np.random.default_rng(0)
    sc = 1.0 / np.sqrt(D)
    inputs = {
        "inp": rng.standard_normal((B, S, D)).astype(np.float32),
        "wq": (rng.standard_normal((D, D)) * sc).astype(np.float32),
        "wk": (rng.standard_normal((D, D)) * sc).astype(np.float32),
        "wv": (rng.standard_normal((D, D)) * sc).astype(np.float32),
        "wo": (rng.standard_normal((D, D)) * sc).astype(np.float32),
        "attn_bias": rng.standard_normal((1, 1, S, S)).astype(np.float32),
        "softcap": 30,
    }
    out = kernel(**inputs)
    print("out", out.shape, out.dtype, float(np.abs(out).max()))


# revision 21
# speedup vs baseline: 1.0906x; 1.0906x over previous
"""Multi-head self-attention with SDPA softcap, sharded over 8 NeuronCores.

Sharding: tensor-parallel over heads. Each core owns 2 of the 16 heads.
Single fused pass: the q/k/v projections for batch 1 are interleaved with
attention units of batch 0, so the activation/vector engines (softmax
tanh+exp, bias adds) stay busy under the projection matmuls.

Dtypes: fp16 for x/weights/q/k/bias/s (11-bit mantissa keeps softmax
logits accurate), bf16 for v and exp(scores) (range), fp32 PSUM/output.
"""

import sys

if "/opt/trn_rl_repo" not in sys.path:
    sys.path.insert(0, "/opt/trn_rl_repo")

import numpy as np

import concourse.bass as bass
import concourse.bacc as bacc
import concourse.tile as tile
from concourse import bass_isa, mybir
from concourse.bass_utils import run_bass_kernel_spmd

F32 = mybir.dt.float32
F32R = mybir.dt.float32r
BF16 = mybir.dt.bfloat16
F16 = mybir.dt.float16

D = 2048          # model dim
H = 16            # total heads
DK = 128          # head dim
B = 2
S = 2048
T = B * S         # 4096 total tokens
NCORES = 8
HC = 2            # heads per core
DPC = HC * DK     # 256: d' slice per core

KC = D // 128     # 16 contraction chunks over model dim
KH = KC // 2      # 8: half of the contraction chunks
TCOL = 512        # phase-1 token-column width
NTCOL = T // TCOL             # 8
TQ = 256          # query-column width per attention unit
NTQ = S // TQ                 # 8 per batch
NTK = S // 128    # 16 key blocks per batch
NBT = T // 128    # 32 token blocks total
NHF = NTK // 2    # 8: half of the key blocks


def _build_program(cap: float):
    nc = bacc.Bacc("TRN2", target_bir_lowering=False, debug=False,
                   num_devices=NCORES)

    xT = nc.dram_tensor("xT", [D, T], F16, kind="ExternalInput").ap()
    ones_d = nc.dram_tensor("ones", [128, 128], BF16, kind="ExternalInput").ap()
    wqT = nc.dram_tensor("wqT", [D, DPC], F16, kind="ExternalInput").ap()
    wkT = nc.dram_tensor("wkT", [D, DPC], F16, kind="ExternalInput").ap()
    wvT = nc.dram_tensor("wvT", [D, DPC], F16, kind="ExternalInput").ap()
    woT = nc.dram_tensor("woT", [DPC, D], F16, kind="ExternalInput").ap()
    biasT = nc.dram_tensor("biasT", [S, S], F16, kind="ExternalInput").ap()
    out_d = nc.dram_tensor("out_partial", [T, D], F32, kind="ExternalOutput").ap()

    xT_v = xT.rearrange("(kc p) t -> p kc t", p=128)
    biasT_v = biasT.rearrange("(kc p) t -> p kc t", p=128)

    with tile.TileContext(nc) as tc:
        with (
            tc.tile_pool(name="const", bufs=1) as cpool,
            tc.tile_pool(name="pqkv", bufs=1) as pqkv,
            tc.tile_pool(name="pwo", bufs=1) as pwo,
            tc.tile_pool(name="pbias", bufs=3) as pbias,
            tc.tile_pool(name="p1w", bufs=1) as p1w,
            tc.tile_pool(name="p1x", bufs=3) as p1x,
            tc.tile_pool(name="p2s", bufs=3) as p2s,
            tc.tile_pool(name="p2er", bufs=3) as p2er,
            tc.tile_pool(name="p2rec", bufs=2) as p2rec,
            tc.tile_pool(name="p2ot", bufs=6) as p2ot,
            tc.tile_pool(name="p2out", bufs=4) as p2out,
            tc.tile_pool(name="psps", bufs=2, space="PSUM") as psps,
            tc.tile_pool(name="pacc", bufs=2, space="PSUM") as pacc,
            tc.tile_pool(name="pio", bufs=2, space="PSUM") as pio,
        ):
            ones_sb = cpool.tile([128, 128], BF16)
            # q/k stored transposed per head: [dk, tokens]; v natural:
            # [token-block, token%128, (h dk)]
            q_sb = pqkv.tile([128, HC, T], F16)
            k_sb = pqkv.tile([128, HC, T], F16)
            v_sb = pqkv.tile([128, NBT, DPC], BF16)
            wo_sb = pwo.tile([128, HC, 4, 512], F16)

            wq_sb = p1w.tile([128, KC, DPC], F16)
            wk_sb = p1w.tile([128, KC, DPC], F16)
            wv_sb = p1w.tile([128, KC, DPC], F16)
            wqv = wqT.rearrange("(kc p) n -> p kc n", p=128)
            nc.sync.dma_start(out=wq_sb[:, 0:KH, :], in_=wqv[:, 0:KH, :])
            nc.scalar.dma_start(out=wq_sb[:, KH:KC, :], in_=wqv[:, KH:KC, :])
            first_x = []
            t0 = 0
            xa0 = p1x.tile([128, KH, TCOL], F16, tag="x")
            xb0 = p1x.tile([128, KH, TCOL], F16, tag="x")
            nc.sync.dma_start(out=xa0[:], in_=xT_v[:, 0:KH, 0:TCOL])
            nc.scalar.dma_start(out=xb0[:], in_=xT_v[:, KH:KC, 0:TCOL])
            first_x.append((xa0, xb0))
            nc.gpsimd.dma_start(
                out=wk_sb[:], in_=wkT.rearrange("(kc p) n -> p kc n", p=128))
            nc.gpsimd.dma_start(
                out=wv_sb[:], in_=wvT.rearrange("(kc p) n -> p kc n", p=128))
            nc.gpsimd.dma_start(out=ones_sb[:], in_=ones_d[:])

            bias_tiles = {}

            def load_bias(b, tqc):
                key = (b, tqc)
                if key in bias_tiles or tqc >= NTQ or b >= B:
                    return
                bt = pbias.tile([128, NTK, TQ], F16, tag="bias")
                nc.gpsimd.dma_start(
                    out=bt[:], in_=biasT_v[:, :, tqc * TQ:(tqc + 1) * TQ])
                bias_tiles[key] = bt

            ncopy = 0

            def proj_tcol(tcol):
                """Emit q/k/v projection work for one 512-token column."""
                nonlocal ncopy
                t0 = tcol * TCOL
                if first_x:
                    xa, xb = first_x.pop()
                else:
                    xa = p1x.tile([128, KH, TCOL], F16, tag="x")
                    xb = p1x.tile([128, KH, TCOL], F16, tag="x")
                    nc.sync.dma_start(
                        out=xa[:], in_=xT_v[:, 0:KH, t0:t0 + TCOL])
                    nc.scalar.dma_start(
                        out=xb[:], in_=xT_v[:, KH:KC, t0:t0 + TCOL])

                # q/k: stationary weights, transposed output [dk, tokens]
                for m in range(4):
                    wsb = wq_sb if m < 2 else wk_sb
                    msl = m % 2
                    dst = q_sb if m < 2 else k_sb
                    ps = pio.tile([128, TCOL], F32, tag="io", name="pqk")
                    for kc in range(KC):
                        xc = xa if kc < KH else xb
                        nc.tensor.matmul(
                            ps[:],
                            wsb[:, kc, msl * 128:(msl + 1) * 128],
                            xc[:, kc % KH, :],
                            start=(kc == 0),
                            stop=(kc == KC - 1),
                        )
                    if ncopy % 3 == 0:
                        nc.scalar.copy(dst[:, msl, t0:t0 + TCOL], ps[:])
                    else:
                        nc.vector.tensor_copy(dst[:, msl, t0:t0 + TCOL], ps[:])
                    ncopy += 1
                # v: stationary x chunks -> natural [t, (h d')] layout
                for tsub in range(TCOL // 128):
                    pv = pacc.tile([128, DPC], F32, tag="acc", name="pv")
                    for kc in range(KC):
                        xc = xa if kc < KH else xb
                        nc.tensor.matmul(
                            pv[:],
                            xc[:, kc % KH, tsub * 128:(tsub + 1) * 128],
                            wv_sb[:, kc, :],
                            start=(kc == 0),
                            stop=(kc == KC - 1),
                        )
                    if ncopy % 3 == 0:
                        nc.scalar.copy(
                            v_sb[:, tcol * (TCOL // 128) + tsub, :], pv[:])
                    else:
                        nc.vector.tensor_copy(
                            v_sb[:, tcol * (TCOL // 128) + tsub, :], pv[:])
                    ncopy += 1

            # units: batch-major so batch-0 units can interleave with the
            # batch-1 projection columns.
            units = [(b, tqc, h)
                     for b in range(B)
                     for tqc in range(NTQ)
                     for h in range(HC)]
            s_map = {}
            fin_state = {}
            ot_map = {}
            nout = 0

            def stage_a(i):
                b, tqc, h = units[i]
                if h == 0 and i + 4 < len(units):
                    load_bias(units[i + 4][0], units[i + 4][1])
                bt = bias_tiles[(b, tqc)]
                q0 = tqc * TQ
                qcol = q_sb[:, h, b * S + q0:b * S + q0 + TQ]
                s_buf = p2s.tile([128, NTK, TQ], F16, tag="s")
                for g in range(NTK // 4):
                    sps = psps.tile([128, 4, TQ], F32, tag="sps")
                    for j in range(4):
                        tkb = g * 4 + j
                        nc.tensor.matmul(
                            sps[:, j, :],
                            k_sb[:, h, b * S + tkb * 128:
                                 b * S + (tkb + 1) * 128],
                            qcol,
                            start=True,
                            stop=True,
                        )
                    nc.vector.tensor_add(
                        s_buf[:, g * 4:(g + 1) * 4, :],
                        sps[:],
                        bt[:, g * 4:(g + 1) * 4, :],
                    )
                s_map[i] = s_buf

            def stage_b(i):
                b, tqc, h = units[i]
                s_buf = s_map.pop(i)
                er = p2er.tile([128, NTK, TQ], BF16, tag="er")
                av = pacc.tile([128, TQ], F32, tag="acc", name="av")
                s_flat = s_buf[:].rearrange("p a b -> p (a b)")
                nc.scalar.activation(
                    s_flat, s_flat,
                    mybir.ActivationFunctionType.Tanh,
                    scale=1.0 / cap,
                )
                nc.scalar.activation(
                    er[:].rearrange("p a b -> p (a b)"),
                    s_flat,
                    mybir.ActivationFunctionType.Exp,
                    scale=cap,
                )
                zp = pio.tile([128, TQ], F32, tag="io", name="zp")
                for tkb in range(NTK):
                    nc.tensor.matmul(
                        av[:],
                        v_sb[:, b * NTK + tkb, h * DK:(h + 1) * DK],
                        er[:, tkb, :],
                        start=(tkb == 0),
                        stop=(tkb == NTK - 1),
                    )
                    nc.tensor.matmul(
                        zp[:],
                        ones_sb[:],
                        er[:, tkb, :],
                        start=(tkb == 0),
                        stop=(tkb == NTK - 1),
                    )
                fin_state[i] = (av, zp)

            def stage_b_fin(i):
                b, tqc, h = units[i]
                av, zp = fin_state.pop(i)
                rec = p2rec.tile([128, TQ], F32, tag="rec")
                nc.vector.reciprocal_approx_fast(out=rec[:], in_=zp[:])
                ot = p2ot.tile([128, TQ], F16, tag="ot")
                nc.vector.tensor_mul(ot[:], av[:], rec[:])
                ot_map[(b, tqc, h)] = ot

            def phase3(b, tqc):
                nonlocal nout
                o0 = ot_map.pop((b, tqc, 0))
                o1 = ot_map.pop((b, tqc, 1))
                for tb4 in range(TQ // 128):
                    trow = b * S + (tqc * (TQ // 128) + tb4) * 128
                    for ng in range(4):
                        po = pio.tile([128, 512], F32, tag="io", name="po")
                        for hc, o in ((0, o0), (1, o1)):
                            nc.tensor.matmul(
                                po[:],
                                o[:, tb4 * 128:(tb4 + 1) * 128],
                                wo_sb[:, hc, ng, :],
                                start=(hc == 0),
                                stop=(hc == HC - 1),
                            )
                        outt = p2out.tile([128, 512], F32, tag="outt")
                        if nout % 3 == 2:
                            nc.scalar.copy(outt[:], po[:])
                        else:
                            nc.vector.tensor_copy(outt[:], po[:])
                        nout += 1
                        nc.sync.dma_start(
                            out=out_d[trow:trow + 128,
                                      ng * 512:(ng + 1) * 512],
                            in_=outt[:],
                        )

            # ---------------- fused emission schedule --------------------
            steps_done = 0

            def unit_steps(n):
                """Advance the unit pipeline by n pipeline steps."""
                nonlocal steps_done
                for _ in range(n):
                    i = steps_done
                    if i >= len(units):
                        return
                    if i == 0:
                        stage_a(0)
                        stage_a(1)
                    stage_b(i)
                    if i + 2 < len(units):
                        stage_a(i + 2)
                    stage_b_fin(i)
                    b, tqc, h = units[i]
                    if h == 1:
                        phase3(b, tqc)
                    steps_done += 1

            proj_tcol(0)
            load_bias(0, 0)
            proj_tcol(1)
            load_bias(0, 1)
            proj_tcol(2)
            nc.gpsimd.dma_start(
                out=wo_sb[:],
                in_=woT.rearrange("(hc p) (ng n) -> p hc ng n", p=128, n=512),
            )
            proj_tcol(3)
            for tcol in range(4, NTCOL):
                proj_tcol(tcol)
                unit_steps(4)
            unit_steps(len(units) - steps_done)

    nc.compile()
    return nc


_PROGRAM_CACHE: dict = {}


def _get_program(cap: float):
    if cap not in _PROGRAM_CACHE:
        _PROGRAM_CACHE[cap] = _build_program(cap)
    return _PROGRAM_CACHE[cap]


def _prepare_in_maps(inp, wq, wk, wv, wo, attn_bias, softcap):
    x = np.ascontiguousarray(np.asarray(inp, dtype=np.float32)).reshape(T, D)
    xT = np.ascontiguousarray(x.T).astype(np.float16)
    biasT = np.ascontiguousarray(
        np.asarray(attn_bias, dtype=np.float32).reshape(S, S).T
    ).astype(np.float16)
    wq = np.asarray(wq, dtype=np.float32)
    wk = np.asarray(wk, dtype=np.float32)
    wv = np.asarray(wv, dtype=np.float32)
    wo = np.asarray(wo, dtype=np.float32)
    scale = 1.0 / np.sqrt(np.float32(DK))
    import ml_dtypes
    ones = np.ones((128, 128), dtype=np.float32).astype(ml_dtypes.bfloat16)

    in_maps = []
    for c in range(NCORES):
        rows = slice(c * DPC, (c + 1) * DPC)
        in_maps.append({
            "xT": xT,
            "ones": ones,
            "wqT": (wq[rows] * scale).T.astype(np.float16),
            "wkT": np.ascontiguousarray(wk[rows].T).astype(np.float16),
            "wvT": np.ascontiguousarray(wv[rows].T).astype(np.float16),
            "woT": np.ascontiguousarray(wo[:, rows].T).astype(np.float16),
            "biasT": biasT,
        })
    return in_maps


def run(inputs: dict, trace: bool = False):
    """Run the SPMD kernel. Returns (full_output, BassKernelResults)."""
    cap = float(inputs["softcap"])
    nc = _get_program(cap)
    in_maps = _prepare_in_maps(
        inputs["inp"], inputs["wq"], inputs["wk"], inputs["wv"],
        inputs["wo"], inputs["attn_bias"], inputs["softcap"],
    )
    res = run_bass_kernel_spmd(
        nc, in_maps, list(range(NCORES)), trace=trace,
    )
    acc = np.zeros((T, D), dtype=np.float32)
    for c in range(NCORES):
        acc += res.results[c]["out_partial"]
    out = acc.reshape(B, S, D)
    return out, res


def kernel(**inputs) -> np.ndarray:
    out, _ = run(inputs, trace=False)
    return out


if __name__ == "__main__":
    rng = np.random.default_rng(0)
    sc = 1.0 / np.sqrt(D)
    inputs = {
        "inp": rng.standard_normal((B, S, D)).astype(np.float32),
        "wq": (rng.standard_normal((D, D)) * sc).astype(np.float32),
        "wk": (rng.standard_normal((D, D)) * sc).astype(np.float32),
        "wv": (rng.standard_normal((D, D)) * sc).astype(np.float32),
        "wo": (rng.standard_normal((D, D)) * sc).astype(np.float32),
        "attn_bias": rng.standard_normal((1, 1, S, S)).astype(np.float32),
        "softcap": 30,
    }
    out = kernel(**inputs)
    print("out", out.shape, out.dtype, float(np.abs(out).max()))


# revision 22
# speedup vs baseline: 1.1707x; 1.0734x over previous
"""Multi-head self-attention with SDPA softcap, sharded over 8 NeuronCores.

Sharding: tensor-parallel over heads. Each core owns 2 of the 16 heads.
Single fused pass: the q/k/v projections for batch 1 are interleaved with
attention units of batch 0, so the activation/vector engines (softmax
tanh+exp, bias adds) stay busy under the projection matmuls.

Dtypes: fp16 for x/weights/q/k/bias/s (11-bit mantissa keeps softmax
logits accurate), bf16 for v and exp(scores) (range), fp32 PSUM/output.
"""

import sys

if "/opt/trn_rl_repo" not in sys.path:
    sys.path.insert(0, "/opt/trn_rl_repo")

import numpy as np

import concourse.bass as bass
import concourse.bacc as bacc
import concourse.tile as tile
from concourse import mybir
from concourse.bass_utils import run_bass_kernel_spmd

F32 = mybir.dt.float32
F32R = mybir.dt.float32r
BF16 = mybir.dt.bfloat16
F16 = mybir.dt.float16

D = 2048          # model dim
H = 16            # total heads
DK = 128          # head dim
B = 2
S = 2048
T = B * S         # 4096 total tokens
NCORES = 8
HC = 2            # heads per core
DPC = HC * DK     # 256: d' slice per core

KC = D // 128     # 16 contraction chunks over model dim
KH = KC // 2      # 8: half of the contraction chunks
TCOL = 512        # phase-1 token-column width
NTCOL = T // TCOL             # 8
TQ = 256          # query-column width per attention unit
NTQ = S // TQ                 # 8 per batch
NTK = S // 128    # 16 key blocks per batch
NBT = T // 128    # 32 token blocks total
NHF = NTK // 2    # 8: half of the key blocks


def _build_program(cap: float):
    nc = bacc.Bacc("TRN2", target_bir_lowering=False, debug=False,
                   num_devices=NCORES)

    xT = nc.dram_tensor("xT", [D, T], F16, kind="ExternalInput").ap()
    ones_d = nc.dram_tensor("ones", [128, 128], BF16, kind="ExternalInput").ap()
    wqT = nc.dram_tensor("wqT", [D, DPC], F16, kind="ExternalInput").ap()
    wkT = nc.dram_tensor("wkT", [D, DPC], F16, kind="ExternalInput").ap()
    wvT = nc.dram_tensor("wvT", [D, DPC], F16, kind="ExternalInput").ap()
    woT = nc.dram_tensor("woT", [DPC, D], F16, kind="ExternalInput").ap()
    biasT = nc.dram_tensor("biasT", [S, S], F16, kind="ExternalInput").ap()
    out_d = nc.dram_tensor("out_partial", [T, D], F32, kind="ExternalOutput").ap()

    xT_v = xT.rearrange("(kc p) t -> p kc t", p=128)
    biasT_v = biasT.rearrange("(kc p) t -> p kc t", p=128)

    with tile.TileContext(nc) as tc:
        with (
            tc.tile_pool(name="const", bufs=1) as cpool,
            tc.tile_pool(name="pqkv", bufs=1) as pqkv,
            tc.tile_pool(name="pwo", bufs=1) as pwo,
            tc.tile_pool(name="pbias", bufs=3) as pbias,
            tc.tile_pool(name="p1w", bufs=1) as p1w,
            tc.tile_pool(name="p1x", bufs=4) as p1x,
            tc.tile_pool(name="p2s", bufs=3) as p2s,
            tc.tile_pool(name="p2er", bufs=3) as p2er,
            tc.tile_pool(name="p2rec", bufs=2) as p2rec,
            tc.tile_pool(name="p2ot", bufs=6) as p2ot,
            tc.tile_pool(name="p2out", bufs=4) as p2out,
            tc.tile_pool(name="psps", bufs=2, space="PSUM") as psps,
            tc.tile_pool(name="pacc", bufs=2, space="PSUM") as pacc,
            tc.tile_pool(name="pio", bufs=2, space="PSUM") as pio,
        ):
            ones_sb = cpool.tile([128, 128], BF16)
            # q/k stored transposed per head: [dk, tokens]; v natural:
            # [token-block, token%128, (h dk)]
            q_sb = pqkv.tile([128, HC, T], F16)
            k_sb = pqkv.tile([128, HC, T], F16)
            v_sb = pqkv.tile([128, NBT, DPC], BF16)
            wo_sb = pwo.tile([128, HC, 4, 512], F16)

            wq_sb = p1w.tile([128, KC, DPC], F16)
            wk_sb = p1w.tile([128, KC, DPC], F16)
            wv_sb = p1w.tile([128, KC, DPC], F16)
            wqv = wqT.rearrange("(kc p) n -> p kc n", p=128)
            nc.sync.dma_start(out=wq_sb[:, 0:KH, :], in_=wqv[:, 0:KH, :])
            nc.scalar.dma_start(out=wq_sb[:, KH:KC, :], in_=wqv[:, KH:KC, :])
            first_x = []
            xa0 = p1x.tile([128, KH, TCOL], F16, tag="x")
            xb0 = p1x.tile([128, KH, TCOL], F16, tag="x")
            nc.sync.dma_start(out=xa0[:], in_=xT_v[:, 0:KH, 0:TCOL])
            nc.scalar.dma_start(out=xb0[:], in_=xT_v[:, KH:KC, 0:TCOL])
            first_x.append((xa0, xb0))
            nc.gpsimd.dma_start(
                out=wk_sb[:], in_=wkT.rearrange("(kc p) n -> p kc n", p=128))
            nc.gpsimd.dma_start(
                out=wv_sb[:], in_=wvT.rearrange("(kc p) n -> p kc n", p=128))
            nc.gpsimd.dma_start(out=ones_sb[:], in_=ones_d[:])

            bias_tiles = {}

            def load_bias(b, tqc):
                key = (b, tqc)
                if key in bias_tiles or tqc >= NTQ or b >= B:
                    return
                bt = pbias.tile([128, NTK, TQ], F16, tag="bias")
                nc.gpsimd.dma_start(
                    out=bt[:], in_=biasT_v[:, :, tqc * TQ:(tqc + 1) * TQ])
                bias_tiles[key] = bt

            ncopy = 0

            def proj_tcol(tcol):
                """Emit q/k/v projection work for one 512-token column."""
                nonlocal ncopy
                t0 = tcol * TCOL
                if first_x:
                    xa, xb = first_x.pop()
                else:
                    xa = p1x.tile([128, KH, TCOL], F16, tag="x")
                    xb = p1x.tile([128, KH, TCOL], F16, tag="x")
                    nc.sync.dma_start(
                        out=xa[:], in_=xT_v[:, 0:KH, t0:t0 + TCOL])
                    nc.scalar.dma_start(
                        out=xb[:], in_=xT_v[:, KH:KC, t0:t0 + TCOL])

                # q/k: stationary weights, transposed output [dk, tokens]
                for m in range(4):
                    wsb = wq_sb if m < 2 else wk_sb
                    msl = m % 2
                    dst = q_sb if m < 2 else k_sb
                    ps = pio.tile([128, TCOL], F32, tag="io", name="pqk")
                    for kc in range(KC):
                        xc = xa if kc < KH else xb
                        nc.tensor.matmul(
                            ps[:],
                            wsb[:, kc, msl * 128:(msl + 1) * 128],
                            xc[:, kc % KH, :],
                            start=(kc == 0),
                            stop=(kc == KC - 1),
                        )
                    if ncopy % 2 == 0:
                        nc.scalar.copy(dst[:, msl, t0:t0 + TCOL], ps[:])
                    else:
                        nc.vector.tensor_copy(dst[:, msl, t0:t0 + TCOL], ps[:])
                    ncopy += 1
                # v: stationary x chunks -> natural [t, (h d')] layout
                for tsub in range(TCOL // 128):
                    pv = pacc.tile([128, DPC], F32, tag="acc", name="pv")
                    for kc in range(KC):
                        xc = xa if kc < KH else xb
                        nc.tensor.matmul(
                            pv[:],
                            xc[:, kc % KH, tsub * 128:(tsub + 1) * 128],
                            wv_sb[:, kc, :],
                            start=(kc == 0),
                            stop=(kc == KC - 1),
                        )
                    if ncopy % 2 == 0:
                        nc.scalar.copy(
                            v_sb[:, tcol * (TCOL // 128) + tsub, :], pv[:])
                    else:
                        nc.vector.tensor_copy(
                            v_sb[:, tcol * (TCOL // 128) + tsub, :], pv[:])
                    ncopy += 1

            # units: batch-major so batch-0 units can interleave with the
            # batch-1 projection columns.
            units = [(b, tqc, h)
                     for b in range(B)
                     for tqc in range(NTQ)
                     for h in range(HC)]
            s_map = {}
            fin_state = {}
            ot_map = {}
            nout = 0

            def stage_a(i):
                b, tqc, h = units[i]
                if h == 0 and i + 4 < len(units):
                    load_bias(units[i + 4][0], units[i + 4][1])
                bt = bias_tiles[(b, tqc)]
                q0 = tqc * TQ
                qcol = q_sb[:, h, b * S + q0:b * S + q0 + TQ]
                s_buf = p2s.tile([128, NTK, TQ], F16, tag="s")
                for g in range(NTK // 4):
                    sps = psps.tile([128, 4, TQ], F32, tag="sps")
                    for j in range(4):
                        tkb = g * 4 + j
                        nc.tensor.matmul(
                            sps[:, j, :],
                            k_sb[:, h, b * S + tkb * 128:
                                 b * S + (tkb + 1) * 128],
                            qcol,
                            start=True,
                            stop=True,
                        )
                    nc.vector.tensor_add(
                        s_buf[:, g * 4:(g + 1) * 4, :],
                        sps[:],
                        bt[:, g * 4:(g + 1) * 4, :],
                    )
                s_map[i] = s_buf

            def stage_b(i):
                b, tqc, h = units[i]
                s_buf = s_map.pop(i)
                er = p2er.tile([128, NTK, TQ], BF16, tag="er")
                av = pacc.tile([128, TQ], F32, tag="acc", name="av")
                zp = pio.tile([128, TQ], F32, tag="io", name="zp")
                s_flat = s_buf[:].rearrange("p a b -> p (a b)")
                nc.scalar.activation(
                    s_flat, s_flat,
                    mybir.ActivationFunctionType.Tanh,
                    scale=1.0 / cap,
                )
                nc.scalar.activation(
                    er[:].rearrange("p a b -> p (a b)"),
                    s_flat,
                    mybir.ActivationFunctionType.Exp,
                    scale=cap,
                )
                for tkb in range(NTK):
                    nc.tensor.matmul(
                        av[:],
                        v_sb[:, b * NTK + tkb, h * DK:(h + 1) * DK],
                        er[:, tkb, :],
                        start=(tkb == 0),
                        stop=(tkb == NTK - 1),
                    )
                    nc.tensor.matmul(
                        zp[:],
                        ones_sb[:],
                        er[:, tkb, :],
                        start=(tkb == 0),
                        stop=(tkb == NTK - 1),
                    )
                fin_state[i] = (av, zp)

            def stage_b_fin(i):
                b, tqc, h = units[i]
                av, zp = fin_state.pop(i)
                rec = p2rec.tile([128, TQ], F32, tag="rec")
                nc.vector.reciprocal_approx_fast(out=rec[:], in_=zp[:])
                ot = p2ot.tile([128, TQ], F16, tag="ot")
                nc.vector.tensor_mul(ot[:], av[:], rec[:])
                ot_map[(b, tqc, h)] = ot

            def phase3(b, tqc):
                nonlocal nout
                o0 = ot_map.pop((b, tqc, 0))
                o1 = ot_map.pop((b, tqc, 1))
                for tb4 in range(TQ // 128):
                    trow = b * S + (tqc * (TQ // 128) + tb4) * 128
                    for ng in range(4):
                        po = pio.tile([128, 512], F32, tag="io", name="po")
                        for hc, o in ((0, o0), (1, o1)):
                            nc.tensor.matmul(
                                po[:],
                                o[:, tb4 * 128:(tb4 + 1) * 128],
                                wo_sb[:, hc, ng, :],
                                start=(hc == 0),
                                stop=(hc == HC - 1),
                            )
                        outt = p2out.tile([128, 512], F32, tag="outt")
                        if nout % 2 == 0:
                            nc.vector.tensor_copy(outt[:], po[:])
                        else:
                            nc.scalar.copy(outt[:], po[:])
                        nout += 1
                        nc.sync.dma_start(
                            out=out_d[trow:trow + 128,
                                      ng * 512:(ng + 1) * 512],
                            in_=outt[:],
                        )

            # ---------------- fused emission schedule --------------------
            steps_done = 0

            def unit_steps(n):
                """Advance the unit pipeline by n pipeline steps."""
                nonlocal steps_done
                for _ in range(n):
                    i = steps_done
                    if i >= len(units):
                        return
                    if i == 0:
                        stage_a(0)
                        stage_a(1)
                    stage_b(i)
                    if i + 2 < len(units):
                        stage_a(i + 2)
                    stage_b_fin(i)
                    b, tqc, h = units[i]
                    if h == 1:
                        phase3(b, tqc)
                    steps_done += 1

            proj_tcol(0)
            load_bias(0, 0)
            proj_tcol(1)
            load_bias(0, 1)
            proj_tcol(2)
            nc.gpsimd.dma_start(
                out=wo_sb[:],
                in_=woT.rearrange("(hc p) (ng n) -> p hc ng n", p=128, n=512),
            )
            proj_tcol(3)
            for tcol in range(4, NTCOL):
                proj_tcol(tcol)
                unit_steps(4)
            unit_steps(len(units) - steps_done)

    nc.compile()
    return nc


_PROGRAM_CACHE: dict = {}


def _get_program(cap: float):
    if cap not in _PROGRAM_CACHE:
        _PROGRAM_CACHE[cap] = _build_program(cap)
    return _PROGRAM_CACHE[cap]


def _prepare_in_maps(inp, wq, wk, wv, wo, attn_bias, softcap):
    x = np.ascontiguousarray(np.asarray(inp, dtype=np.float32)).reshape(T, D)
    xT = np.ascontiguousarray(x.T).astype(np.float16)
    biasT = np.ascontiguousarray(
        np.asarray(attn_bias, dtype=np.float32).reshape(S, S).T
    ).astype(np.float16)
    wq = np.asarray(wq, dtype=np.float32)
    wk = np.asarray(wk, dtype=np.float32)
    wv = np.asarray(wv, dtype=np.float32)
    wo = np.asarray(wo, dtype=np.float32)
    scale = 1.0 / np.sqrt(np.float32(DK))
    import ml_dtypes
    ones = np.ones((128, 128), dtype=np.float32).astype(ml_dtypes.bfloat16)

    in_maps = []
    for c in range(NCORES):
        rows = slice(c * DPC, (c + 1) * DPC)
        in_maps.append({
            "xT": xT,
            "ones": ones,
            "wqT": (wq[rows] * scale).T.astype(np.float16),
            "wkT": np.ascontiguousarray(wk[rows].T).astype(np.float16),
            "wvT": np.ascontiguousarray(wv[rows].T).astype(np.float16),
            "woT": np.ascontiguousarray(wo[:, rows].T).astype(np.float16),
            "biasT": biasT,
        })
    return in_maps


def run(inputs: dict, trace: bool = False):
    """Run the SPMD kernel. Returns (full_output, BassKernelResults)."""
    cap = float(inputs["softcap"])
    nc = _get_program(cap)
    in_maps = _prepare_in_maps(
        inputs["inp"], inputs["wq"], inputs["wk"], inputs["wv"],
        inputs["wo"], inputs["attn_bias"], inputs["softcap"],
    )
    res = run_bass_kernel_spmd(
        nc, in_maps, list(range(NCORES)), trace=trace,
    )
    acc = np.zeros((T, D), dtype=np.float32)
    for c in range(NCORES):
        acc += res.results[c]["out_partial"]
    out = acc.reshape(B, S, D)
    return out, res


def kernel(**inputs) -> np.ndarray:
    out, _ = run(inputs, trace=False)
    return out


if __name__ == "__main__":
    rng = np.random.default_rng(0)
    sc = 1.0 / np.sqrt(D)
    inputs = {
        "inp": rng.standard_normal((B, S, D)).astype(np.float32),
        "wq": (rng.standard_normal((D, D)) * sc).astype(np.float32),
        "wk": (rng.standard_normal((D, D)) * sc).astype(np.float32),
        "wv": (rng.standard_normal((D, D)) * sc).astype(np.float32),
        "wo": (rng.standard_normal((D, D)) * sc).astype(np.float32),
        "attn_bias": rng.standard_normal((1, 1, S, S)).astype(np.float32),
        "softcap": 30,
    }
    out = kernel(**inputs)
    print("out", out.shape, out.dtype, float(np.abs(out).max()))
